# revision 24
# baseline (speedup 1.0000x reference)
"""Causal self-attention (B=4, T=2048, C=1024, H=16) on 8 trn2 NeuronCores.

Sharding: core c = (batch b = c//2, head-group g = c%2). Each core computes
the full attention for batch b and heads 8g..8g+7 (column-parallel qkv,
row-parallel proj), producing a partial [T, C] output (bf16); the host sums
the two partials per batch in fp32.

Per-core device kernel (Bass/Tile, SPMD same program on all 8 cores):
  warmup  dummy matmuls so the PE HAM clock-gate is warm before real work
  qT/kT  [512, T] = (wq|wk).T @ x.T        (bf16 matmuls, fp32 psum)
  v      [T, 8, 65]  (natural layout, ones column appended per head)
  S^T    [tk 128, tq 512] blocks = kT.T-slices @ qT-slices (2 heads row-packed)
  P^T    = exp(S^T/8) then 0/1-triangle multiply on the diagonal tile (DVE)
  y/l    = [v|1].T @ P^T  accumulated over tk  -> [65, tq] psum per head
  1/l    via GpSimd partition_broadcast of l + DVE reciprocal_approx_fast
         (the reciprocal runs on the 64-partition broadcast: it gives wrong
         results on HW for single-partition inputs)
  yT_n   = yT * (1/l)                      (DVE)
  out    = yT_n.T @ wo -> [T, C] bf16 partial

All inputs are pre-arranged on the host into the exact SBUF layout
([partition, chunk, free], contiguous) so every input DMA is a single
large-run transfer. The PE instruction stream is explicitly interleaved:
during attention stage m the projection matmuls of stage m+1 (and the v /
proj streams) are emitted ~2 per attention block so the PE never idles on
ScalarE's exp.
"""

import numpy as np

import concourse.bacc as bacc
import concourse.bass as bass
import concourse.library_config as library_config
import concourse.mybir as mybir
import concourse.tile as tile
from concourse.bass_utils import run_bass_kernel_spmd

try:
    import ml_dtypes

    BF16 = np.dtype(ml_dtypes.bfloat16)
except ImportError:  # pragma: no cover
    BF16 = np.dtype("bfloat16")

B, T, C = 4, 2048, 1024
N_HEAD = 16
D = 64  # head dim
H_LOC = 8  # heads per core
DL = H_LOC * D  # 512, local d width per core
CK = C // 128  # 8 contraction chunks
DT = mybir.dt.bfloat16
F32 = mybir.dt.float32
N_WARM = 40  # warmup matmuls to open the HAM clock gate during input DMA
N_WARM2 = 16  # extra warmups bridging the first DMA-paced prologue waits


def build_program(t_len=T, enable_asserts=False):
    """Build the SPMD per-core program. Returns the compiled Bacc object."""
    NJ = t_len // 512  # tq chunks
    NTT = t_len // 128  # 128-wide t tiles
    MD = DL // 128  # 4 d-chunks of qT/kT/yT

    nc = bacc.Bacc(
        "TRN2",
        target_bir_lowering=False,
        debug=False,
        enable_asserts=enable_asserts,
        num_devices=8,
    )

    x_d = [
        nc.dram_tensor(f"x{q}", [128, CK, 512], DT, kind="ExternalInput").ap()
        for q in range(NJ)
    ]
    wq_d = nc.dram_tensor("wq", [128, CK, DL], DT, kind="ExternalInput").ap()
    wk_d = nc.dram_tensor("wk", [128, CK, DL], DT, kind="ExternalInput").ap()
    wv_d = nc.dram_tensor("wv", [128, CK, DL], DT, kind="ExternalInput").ap()
    wo_d = nc.dram_tensor("wo", [128, MD, C], DT, kind="ExternalInput").ap()
    tri_d = nc.dram_tensor("tri", [128, 128], DT, kind="ExternalInput").ap()
    out_d = nc.dram_tensor("out", [t_len, C], DT, kind="ExternalOutput").ap()

    with tile.TileContext(nc) as tc:
        with (
            tc.tile_pool(name="consts", bufs=1) as cpool,
            tc.tile_pool(name="ptp", bufs=4) as pt_pool,
            tc.tile_pool(name="yup", bufs=3) as yu_pool,
            tc.tile_pool(name="rlp", bufs=3) as rl_pool,
            tc.tile_pool(name="outp", bufs=3) as out_pool,
            tc.tile_pool(name="psum", bufs=1, space="PSUM") as psum,
        ):
            # ---- persistent SBUF tensors ----
            xt_q = [
                cpool.tile([128, CK, 512], DT, name=f"xt{q}") for q in range(NJ)
            ]
            wq_t = cpool.tile([128, CK, DL], DT, name="wqt")
            wk_t = cpool.tile([128, CK, DL], DT, name="wkt")
            wv_t = cpool.tile([128, CK, DL], DT, name="wvt")
            wo_t = cpool.tile([128, MD, C], DT, name="wot")
            qt_t = cpool.tile([128, MD, t_len], DT, name="qtt")
            kt_t = cpool.tile([128, MD, t_len], DT, name="ktt")
            v_t = cpool.tile([128, NTT, H_LOC, D + 1], DT, name="vt")
            yt_t = cpool.tile([128, MD, t_len], DT, name="ytt")
            tri_t = cpool.tile([128, 2, 128], DT, name="trit")
            warm_t = cpool.tile([128, 512], DT, name="warmt")

            # memsets first so the DVE work (warm tile for the PE warmup,
            # ones column) isn't queued behind the vector-issued DMAs
            nc.vector.memset(warm_t[:, :], 0.25)
            nc.vector.memset(v_t[:, :, :, D : D + 1], 1.0)

            # ---- input DMAs: three queues in parallel (sync / scalar /
            # gpsimd are the only DMA-capable engines), each ordered by when
            # compute needs its tensors ----
            nc.sync.dma_start(out=xt_q[0][:, :, :], in_=x_d[0])
            for q in range(1, NJ):
                nc.sync.dma_start(out=xt_q[q][:, :, :], in_=x_d[q])
            nc.scalar.dma_start(out=wq_t[:, :, :], in_=wq_d)
            nc.scalar.dma_start(out=tri_t[:, 0, :], in_=tri_d)
            nc.scalar.dma_start(out=tri_t[:, 1, :], in_=tri_d)
            nc.scalar.dma_start(out=wv_t[:, :, :], in_=wv_d)
            nc.scalar.dma_start(out=wo_t[:, :, :], in_=wo_d)
            nc.gpsimd.dma_start(out=wk_t[:, :, :], in_=wk_d)

            # partition_broadcast ucode lives in the `proxy` library, not the
            # default-resident `standard` one — load it after the wk DMA
            # issue but before any gpsimd compute op
            nc.gpsimd.load_library(library_config.proxy)

            def warmup(n):
                for _ in range(n):
                    wps = psum.tile([128, 512], F32, name="qkvps", bufs=2)
                    nc.tensor.matmul(
                        wps[:, :],
                        lhsT=warm_t[:, 0:128],
                        rhs=warm_t[:, :],
                        start=True,
                        stop=True,
                    )

            # ---- projection step generators (one yield per matmul) ----
            def qk_steps(m, jlist, parts=("q", "k")):
                for part, w_t, dst_t in (
                    ("q", wq_t, qt_t),
                    ("k", wk_t, kt_t),
                ):
                    if part not in parts:
                        continue
                    for j in jlist:
                        ps = psum.tile([128, 512], F32, name="qkvps", bufs=2)
                        for k in range(CK):
                            nc.tensor.matmul(
                                ps[:, :],
                                lhsT=w_t[:, k, 128 * m : 128 * (m + 1)],
                                rhs=xt_q[j][:, k, :],
                                start=(k == 0),
                                stop=(k == CK - 1),
                            )
                            if k < CK - 1:
                                yield
                        nc.vector.tensor_copy(
                            dst_t[:, m, 512 * j : 512 * (j + 1)], ps[:, :]
                        )
                        yield

            def qk_steps_paired(m):
                # two j-chunks per weight chunk: consecutive matmuls share the
                # stationary operand, halving the weight-load traffic
                for w_t, dst_t in ((wq_t, qt_t), (wk_t, kt_t)):
                    for j0 in (0, 2):
                        psA = psum.tile([128, 512], F32, name="qkvps", bufs=2)
                        psB = psum.tile([128, 512], F32, name="qkvps", bufs=2)
                        for k in range(CK):
                            for ps, j in ((psA, j0), (psB, j0 + 1)):
                                nc.tensor.matmul(
                                    ps[:, :],
                                    lhsT=w_t[:, k, 128 * m : 128 * (m + 1)],
                                    rhs=xt_q[j][:, k, :],
                                    start=(k == 0),
                                    stop=(k == CK - 1),
                                )
                                if not (k == CK - 1 and j == j0 + 1):
                                    yield
                        for ps, j in ((psA, j0), (psB, j0 + 1)):
                            nc.vector.tensor_copy(
                                dst_t[:, m, 512 * j : 512 * (j + 1)], ps[:, :]
                            )
                        yield

            def v_steps(t0, t1):
                for ti in range(t0, t1):
                    q, off = ti // 4, 128 * (ti % 4)
                    ps = psum.tile([128, 512], F32, name="qkvps", bufs=2)
                    for k in range(CK):
                        nc.tensor.matmul(
                            ps[:, :],
                            lhsT=xt_q[q][:, k, off : off + 128],
                            rhs=wv_t[:, k, :],
                            start=(k == 0),
                            stop=(k == CK - 1),
                        )
                        if k < CK - 1:
                            yield
                    nc.vector.tensor_copy(
                        v_t[:, ti, :, 0:D],
                        ps[:, :].rearrange("p (h d) -> p h d", h=H_LOC),
                    )
                    yield

            def proj_steps():
                for ti in range(NTT):
                    tt = slice(128 * ti, 128 * (ti + 1))
                    ot = out_pool.tile([128, C], DT, name="ot")
                    for ci in range(2):
                        cs = slice(512 * ci, 512 * (ci + 1))
                        if ti >= 12:
                            # attention psum is free by now; wider rotation
                            # so the final drain isn't cast-latency bound
                            ps = psum.tile([128, 2, 512], F32, name="sps", bufs=2)
                            ps = ps[:, 0, :]
                        else:
                            ps = psum.tile([128, 512], F32, name="qkvps", bufs=2)
                        for hp in range(MD):
                            nc.tensor.matmul(
                                ps[:, :],
                                lhsT=yt_t[:, hp, tt],
                                rhs=wo_t[:, hp, cs],
                                start=(hp == 0),
                                stop=(hp == MD - 1),
                            )
                            if hp < MD - 1:
                                yield
                        nc.vector.tensor_copy(ot[:, cs], ps[:, :])
                        if ci == 0:
                            yield
                    nc.sync.dma_start(out=out_d[tt, :], in_=ot[:, :])
                    yield

            # ---- filler stream: consumed 2 steps per attention block ----
            def filler_gen():
                yield from qk_steps(0, [1])
                yield from v_steps(4, 8)
                yield "v7"
                yield from qk_steps(0, [2])
                yield from v_steps(8, 12)
                yield "v11"
                yield from qk_steps(0, [3])
                yield from v_steps(12, 16)
                yield "v15"
                for m in range(1, MD):
                    yield from qk_steps_paired(m)
                    yield f"qk{m}"

            fill = {"it": filler_gen(), "seen": set(), "done": False}

            def consume(n):
                if fill["done"]:
                    return
                got = 0
                while got < n:
                    try:
                        item = next(fill["it"])
                    except StopIteration:
                        fill["done"] = True
                        return
                    if isinstance(item, str):
                        fill["seen"].add(item)
                    else:
                        got += 1

            def drain(tag):
                if fill["done"] or tag in fill["seen"]:
                    return
                while True:
                    try:
                        item = next(fill["it"])
                    except StopIteration:
                        fill["done"] = True
                        return
                    if isinstance(item, str):
                        fill["seen"].add(item)
                        if item == tag:
                            return

            proj = {"it": None, "done": True, "count": 0}

            def consume_proj(n, cap=None):
                # cap: never emit proj steps for a tile whose yt inputs
                # haven't been emitted yet (would deadlock the PE queue)
                if proj["done"]:
                    return
                got = 0
                while got < n and (cap is None or proj["count"] < cap):
                    try:
                        next(proj["it"])
                        proj["count"] += 1
                        got += 1
                    except StopIteration:
                        proj["done"] = True
                        return

            # ---- attention stage for head-pair hp ----
            def attn_stage(hp):
                for j in range(NJ):
                    if hp == 0 and j >= 1:
                        drain(f"v{4 * j + 3}")
                    tq0 = 512 * j
                    nblk = 4 * j + 4  # causal: tk blocks 0 .. 4j+3
                    accA = psum.tile([D + 1, 512], F32, name="acc", bufs=2)
                    accB = psum.tile([D + 1, 512], F32, name="acc", bufs=2)
                    pend = []  # software pipeline: AV for block i-1 after S of i

                    def flush_av():
                        for mm in pend:
                            nc.tensor.matmul(**mm)
                        pend.clear()

                    for i in range(nblk):
                        tk = slice(128 * i, 128 * (i + 1))
                        diag = i - 4 * j
                        lo = 128 * diag if diag >= 0 else 0
                        tqs = slice(tq0 + lo, tq0 + 512)
                        sps = psum.tile([128, 2, 512], F32, name="sps", bufs=2)
                        for h2, lohi in ((0, slice(0, 64)), (1, slice(64, 128))):
                            nc.tensor.matmul(
                                sps[:, h2, lo:],
                                lhsT=kt_t[lohi, hp, tk],
                                rhs=qt_t[lohi, hp, tqs],
                                start=True,
                                stop=True,
                            )
                        pt = pt_pool.tile([128, 2, 512], DT, name="pt")
                        nc.scalar.activation(
                            pt[:, :, lo:],
                            sps[:, :, lo:],
                            mybir.ActivationFunctionType.Exp,
                            scale=0.125,
                        )
                        if diag >= 0:  # zero the above-diagonal triangle
                            dg = slice(lo, lo + 128)
                            nc.vector.tensor_mul(
                                pt[:, :, dg], pt[:, :, dg], tri_t[:, :, :]
                            )
                        if hp < MD - 1:
                            consume(2)
                        elif j >= 1:
                            # only proj tiles for tq chunks < j have their yt
                            # fully emitted (8 steps per tile, 4 tiles/chunk)
                            consume_proj(3, cap=8 * 4 * j)
                        flush_av()
                        for h2, acc in ((0, accA), (1, accB)):
                            pend.append(
                                dict(
                                    out=acc[:, lo:],
                                    lhsT=v_t[:, i, 2 * hp + h2, :],
                                    rhs=pt[:, h2, lo:],
                                    start=(i == 0),
                                    stop=(i == nblk - 1),
                                )
                            )
                    flush_av()

                    # normalization tail. Order matters: the psum-reading
                    # copies (yu, lb) for BOTH heads go first so the acc psum
                    # banks free quickly for the next j / next stage.
                    tq = slice(tq0, tq0 + 512)
                    yus, lbs = [], []
                    for h2, acc in ((0, accA), (1, accB)):
                        yu = yu_pool.tile([64, 512], DT, name="yu")
                        nc.vector.tensor_copy(yu[:, :], acc[0:D, :])
                        lb = rl_pool.tile([1, 512], F32, name="lb")
                        nc.vector.tensor_copy(lb[:, :], acc[D : D + 1, :])
                        yus.append(yu)
                        lbs.append(lb)
                    for h2 in range(2):
                        lbc = rl_pool.tile([64, 512], F32, name="lbc")
                        nc.gpsimd.partition_broadcast(lbc[:, :], lbs[h2][:, :])
                        rli = rl_pool.tile([64, 512], F32, name="rli")
                        nc.vector.reciprocal_approx_fast(rli[:, :], lbc[:, :])
                        nc.vector.tensor_mul(
                            yt_t[64 * h2 : 64 * (h2 + 1), hp, tq],
                            yus[h2][:, :],
                            rli[:, :],
                        )

            # ---- main pipeline ----
            # prologue ordered by DMA arrival: q(j0) [wq,x0], v(0-3) [wv],
            # k(j0) [wk]; the rest of stage-0 q/k flows through the fillers
            warmup(N_WARM)
            for _ in qk_steps(0, [0], parts=("q",)):
                pass
            warmup(N_WARM2)
            for _ in v_steps(0, 4):
                pass
            for _ in qk_steps(0, [0], parts=("k",)):
                pass

            for hp in range(MD):
                if hp >= 1:
                    drain(f"qk{hp}")
                if hp == MD - 1:
                    proj.update(it=proj_steps(), done=False)
                attn_stage(hp)

            consume_proj(10**9)  # finish remaining projection tiles

    nc.compile()
    return nc


def make_host_inputs(x, w_qkv, w_proj, t_len=T):
    """Shard full inputs into the 8 per-core input dicts.

    Everything is pre-arranged into the on-chip SBUF layout
    [partition, chunk, free] (contiguous) so each DMA is one large-run copy.
    """
    NJ = t_len // 512
    tri = np.where(
        np.arange(128)[None, :] >= np.arange(128)[:, None], 1.0, 0.0
    ).astype(BF16)

    def chunked(w, width):  # [C, width] -> [128, C//128, width] contiguous
        return np.ascontiguousarray(
            w.reshape(C // 128, 128, width).transpose(1, 0, 2)
        ).astype(BF16)

    in_maps = []
    for c in range(8):
        b, g = c // 2, c % 2
        xT = np.ascontiguousarray(x[b][:t_len].T)  # [C, T]
        xq = xT.reshape(CK, 128, t_len)
        m = dict(
            wq=chunked(w_qkv[:, 512 * g : 512 * (g + 1)], DL),
            wk=chunked(w_qkv[:, C + 512 * g : C + 512 * (g + 1)], DL),
            wv=chunked(w_qkv[:, 2 * C + 512 * g : 2 * C + 512 * (g + 1)], DL),
            wo=np.ascontiguousarray(
                w_proj[512 * g : 512 * (g + 1), :]
                .reshape(MD := DL // 128, 128, C)
                .transpose(1, 0, 2)
            ).astype(BF16),
            tri=tri,
        )
        for q in range(NJ):
            m[f"x{q}"] = np.ascontiguousarray(
                xq[:, :, 512 * q : 512 * (q + 1)].transpose(1, 0, 2)
            ).astype(BF16)
        in_maps.append(m)
    return in_maps


_CACHE = {}


def _get_program():
    if "nc" not in _CACHE:
        _CACHE["nc"] = build_program()
    return _CACHE["nc"]


def kernel(x, w_qkv, w_proj, _trace=False, _trace_kwargs=None):
    x = np.asarray(x, np.float32)
    w_qkv = np.asarray(w_qkv, np.float32)
    w_proj = np.asarray(w_proj, np.float32)
    nc = _get_program()
    in_maps = make_host_inputs(x, w_qkv, w_proj)
    kw = {}
    if _trace:
        kw = dict(trace=True, **(_trace_kwargs or {}))
    res = run_bass_kernel_spmd(nc, in_maps, core_ids=list(range(8)), **kw)
    out = np.empty((B, T, C), np.float32)
    for b in range(B):
        out[b] = res.results[2 * b]["out"].astype(np.float32) + res.results[
            2 * b + 1
        ]["out"].astype(np.float32)
    if _trace:
        return out, res
    return out


# revision 25
# speedup vs baseline: 1.0288x; 1.0288x over previous
"""Causal self-attention (B=4, T=2048, C=1024, H=16) on 8 trn2 NeuronCores.

Sharding: core c = (batch b = c//2, head-group g = c%2). Each core computes
the full attention for batch b and heads 8g..8g+7 (column-parallel qkv,
row-parallel proj), producing a partial [T, C] output (bf16); the host sums
the two partials per batch in fp32.

Per-core device kernel (Bass/Tile, SPMD same program on all 8 cores):
  warmup  dummy matmuls so the PE HAM clock-gate is warm before real work
  qT/kT  [512, T] = (wq|wk).T @ x.T        (bf16 matmuls, fp32 psum)
  v      [T, 8, 65]  (natural layout, ones column appended per head)
  S^T    [tk 128, tq 512] blocks = kT.T-slices @ qT-slices (2 heads row-packed)
  P^T    = exp(S^T/8) then 0/1-triangle multiply on the diagonal tile (DVE)
  y/l    = [v|1].T @ P^T  accumulated over tk  -> [65, tq] psum per head
  1/l    via GpSimd partition_broadcast of l + DVE reciprocal_approx_fast
         (the reciprocal runs on the 64-partition broadcast: it gives wrong
         results on HW for single-partition inputs)
  yT_n   = yT * (1/l)                      (DVE)
  out    = yT_n.T @ wo -> [T, C] bf16 partial

All inputs are pre-arranged on the host into the exact SBUF layout
([partition, chunk, free], contiguous) so every input DMA is a single
large-run transfer. The PE instruction stream is explicitly interleaved:
during attention stage m the projection matmuls of stage m+1 (and the v /
proj streams) are emitted ~2 per attention block so the PE never idles on
ScalarE's exp.
"""

import numpy as np

import concourse.bacc as bacc
import concourse.bass as bass
import concourse.library_config as library_config
import concourse.mybir as mybir
import concourse.tile as tile
from concourse.bass_utils import run_bass_kernel_spmd

try:
    import ml_dtypes

    BF16 = np.dtype(ml_dtypes.bfloat16)
except ImportError:  # pragma: no cover
    BF16 = np.dtype("bfloat16")

B, T, C = 4, 2048, 1024
N_HEAD = 16
D = 64  # head dim
H_LOC = 8  # heads per core
DL = H_LOC * D  # 512, local d width per core
CK = C // 128  # 8 contraction chunks
DT = mybir.dt.bfloat16
F32 = mybir.dt.float32
N_WARM = 58  # warmup matmuls to open the HAM clock gate during input DMA
N_WARM2 = 16  # extra warmups bridging the first DMA-paced prologue waits


def build_program(t_len=T, enable_asserts=False):
    """Build the SPMD per-core program. Returns the compiled Bacc object."""
    NJ = t_len // 512  # tq chunks
    NTT = t_len // 128  # 128-wide t tiles
    MD = DL // 128  # 4 d-chunks of qT/kT/yT

    nc = bacc.Bacc(
        "TRN2",
        target_bir_lowering=False,
        debug=False,
        enable_asserts=enable_asserts,
        num_devices=8,
    )

    x_d = [
        nc.dram_tensor(f"x{q}", [128, CK, 512], DT, kind="ExternalInput").ap()
        for q in range(NJ)
    ]
    wq_d = nc.dram_tensor("wq", [128, CK, DL], DT, kind="ExternalInput").ap()
    wk_d = nc.dram_tensor("wk", [128, CK, DL], DT, kind="ExternalInput").ap()
    wv_d = nc.dram_tensor("wv", [128, CK, DL], DT, kind="ExternalInput").ap()
    wo_d = nc.dram_tensor("wo", [128, MD, C], DT, kind="ExternalInput").ap()
    tri_d = nc.dram_tensor("tri", [128, 128], DT, kind="ExternalInput").ap()
    out_d = nc.dram_tensor("out", [t_len, C], DT, kind="ExternalOutput").ap()

    with tile.TileContext(nc) as tc:
        with (
            tc.tile_pool(name="consts", bufs=1) as cpool,
            tc.tile_pool(name="ptp", bufs=6) as pt_pool,
            tc.tile_pool(name="yup", bufs=4) as yu_pool,
            tc.tile_pool(name="rlp", bufs=4) as rl_pool,
            tc.tile_pool(name="outp", bufs=4) as out_pool,
            tc.tile_pool(name="psum", bufs=1, space="PSUM") as psum,
        ):
            # ---- persistent SBUF tensors ----
            xt_q = [
                cpool.tile([128, CK, 512], DT, name=f"xt{q}") for q in range(NJ)
            ]
            wq_t = cpool.tile([128, CK, DL], DT, name="wqt")
            wk_t = cpool.tile([128, CK, DL], DT, name="wkt")
            wv_t = cpool.tile([128, CK, DL], DT, name="wvt")
            wo_t = cpool.tile([128, MD, C], DT, name="wot")
            qt_t = cpool.tile([128, MD, t_len], DT, name="qtt")
            kt_t = cpool.tile([128, MD, t_len], DT, name="ktt")
            v_t = cpool.tile([128, NTT, H_LOC, D + 1], DT, name="vt")
            yt_t = cpool.tile([128, MD, t_len], DT, name="ytt")
            tri_t = cpool.tile([128, 2, 128], DT, name="trit")
            warm_t = cpool.tile([128, 512], DT, name="warmt")

            # memsets first so the DVE work (warm tile for the PE warmup,
            # ones column) isn't queued behind the vector-issued DMAs
            nc.vector.memset(warm_t[:, :], 0.25)
            nc.vector.memset(v_t[:, :, :, D : D + 1], 1.0)

            # ---- input DMAs: three queues in parallel (sync / scalar /
            # gpsimd are the only DMA-capable engines), each ordered by when
            # compute needs its tensors ----
            nc.sync.dma_start(out=xt_q[0][:, :, :], in_=x_d[0])
            for q in range(1, NJ):
                nc.sync.dma_start(out=xt_q[q][:, :, :], in_=x_d[q])
            nc.scalar.dma_start(out=wq_t[:, :, :], in_=wq_d)
            nc.scalar.dma_start(out=tri_t[:, 0, :], in_=tri_d)
            nc.scalar.dma_start(out=tri_t[:, 1, :], in_=tri_d)
            nc.scalar.dma_start(out=wv_t[:, :, :], in_=wv_d)
            nc.scalar.dma_start(out=wo_t[:, :, :], in_=wo_d)
            nc.gpsimd.dma_start(out=wk_t[:, :, :], in_=wk_d)

            # partition_broadcast ucode lives in the `proxy` library, not the
            # default-resident `standard` one — load it after the wk DMA
            # issue but before any gpsimd compute op
            nc.gpsimd.load_library(library_config.proxy)

            def warmup(n):
                for _ in range(n):
                    wps = psum.tile([128, 512], F32, name="qkvps", bufs=2)
                    nc.tensor.matmul(
                        wps[:, :],
                        lhsT=warm_t[:, 0:128],
                        rhs=warm_t[:, :],
                        start=True,
                        stop=True,
                    )

            # ---- projection step generators (one yield per matmul) ----
            def qk_steps(m, jlist, parts=("q", "k")):
                for part, w_t, dst_t in (
                    ("q", wq_t, qt_t),
                    ("k", wk_t, kt_t),
                ):
                    if part not in parts:
                        continue
                    for j in jlist:
                        ps = psum.tile([128, 512], F32, name="qkvps", bufs=2)
                        for k in range(CK):
                            nc.tensor.matmul(
                                ps[:, :],
                                lhsT=w_t[:, k, 128 * m : 128 * (m + 1)],
                                rhs=xt_q[j][:, k, :],
                                start=(k == 0),
                                stop=(k == CK - 1),
                            )
                            if k < CK - 1:
                                yield
                        nc.vector.tensor_copy(
                            dst_t[:, m, 512 * j : 512 * (j + 1)], ps[:, :]
                        )
                        yield

            def qk_steps_paired(m):
                # two j-chunks per weight chunk: consecutive matmuls share the
                # stationary operand, halving the weight-load traffic
                for w_t, dst_t in ((wq_t, qt_t), (wk_t, kt_t)):
                    for j0 in (0, 2):
                        psA = psum.tile([128, 512], F32, name="qkvps", bufs=2)
                        psB = psum.tile([128, 512], F32, name="qkvps", bufs=2)
                        for k in range(CK):
                            for ps, j in ((psA, j0), (psB, j0 + 1)):
                                nc.tensor.matmul(
                                    ps[:, :],
                                    lhsT=w_t[:, k, 128 * m : 128 * (m + 1)],
                                    rhs=xt_q[j][:, k, :],
                                    start=(k == 0),
                                    stop=(k == CK - 1),
                                )
                                if not (k == CK - 1 and j == j0 + 1):
                                    yield
                        for ps, j in ((psA, j0), (psB, j0 + 1)):
                            nc.vector.tensor_copy(
                                dst_t[:, m, 512 * j : 512 * (j + 1)], ps[:, :]
                            )
                        yield

            def v_steps(t0, t1):
                for ti in range(t0, t1):
                    q, off = ti // 4, 128 * (ti % 4)
                    ps = psum.tile([128, 512], F32, name="qkvps", bufs=2)
                    for k in range(CK):
                        nc.tensor.matmul(
                            ps[:, :],
                            lhsT=xt_q[q][:, k, off : off + 128],
                            rhs=wv_t[:, k, :],
                            start=(k == 0),
                            stop=(k == CK - 1),
                        )
                        if k < CK - 1:
                            yield
                    nc.vector.tensor_copy(
                        v_t[:, ti, :, 0:D],
                        ps[:, :].rearrange("p (h d) -> p h d", h=H_LOC),
                    )
                    yield

            def proj_steps():
                for ti in range(NTT):
                    tt = slice(128 * ti, 128 * (ti + 1))
                    ot = out_pool.tile([128, C], DT, name="ot")
                    for ci in range(2):
                        cs = slice(512 * ci, 512 * (ci + 1))
                        if ti >= 12:
                            # attention psum is free by now; wider rotation
                            # so the final drain isn't cast-latency bound
                            ps = psum.tile([128, 2, 512], F32, name="sps", bufs=2)
                            ps = ps[:, 0, :]
                        else:
                            ps = psum.tile([128, 512], F32, name="qkvps", bufs=2)
                        for hp in range(MD):
                            nc.tensor.matmul(
                                ps[:, :],
                                lhsT=yt_t[:, hp, tt],
                                rhs=wo_t[:, hp, cs],
                                start=(hp == 0),
                                stop=(hp == MD - 1),
                            )
                            if hp < MD - 1:
                                yield
                        nc.vector.tensor_copy(ot[:, cs], ps[:, :])
                        if ci == 0:
                            yield
                    nc.sync.dma_start(out=out_d[tt, :], in_=ot[:, :])
                    yield

            # ---- filler stream: consumed 2 steps per attention block ----
            def filler_gen():
                yield from qk_steps(0, [1])
                yield from v_steps(4, 8)
                yield "v7"
                yield from qk_steps(0, [2])
                yield from v_steps(8, 12)
                yield "v11"
                yield from qk_steps(0, [3])
                yield from v_steps(12, 16)
                yield "v15"
                for m in range(1, MD):
                    yield from qk_steps(m, range(NJ))
                    yield f"qk{m}"

            fill = {"it": filler_gen(), "seen": set(), "done": False}

            def consume(n):
                if fill["done"]:
                    return
                got = 0
                while got < n:
                    try:
                        item = next(fill["it"])
                    except StopIteration:
                        fill["done"] = True
                        return
                    if isinstance(item, str):
                        fill["seen"].add(item)
                    else:
                        got += 1

            def drain(tag):
                if fill["done"] or tag in fill["seen"]:
                    return
                while True:
                    try:
                        item = next(fill["it"])
                    except StopIteration:
                        fill["done"] = True
                        return
                    if isinstance(item, str):
                        fill["seen"].add(item)
                        if item == tag:
                            return

            proj = {"it": None, "done": True, "count": 0}

            def consume_proj(n, cap=None):
                # cap: never emit proj steps for a tile whose yt inputs
                # haven't been emitted yet (would deadlock the PE queue)
                if proj["done"]:
                    return
                got = 0
                while got < n and (cap is None or proj["count"] < cap):
                    try:
                        next(proj["it"])
                        proj["count"] += 1
                        got += 1
                    except StopIteration:
                        proj["done"] = True
                        return

            # ---- attention stage for head-pair hp ----
            def attn_stage(hp):
                for j in range(NJ):
                    if hp == 0 and j >= 1:
                        drain(f"v{4 * j + 3}")
                    tq0 = 512 * j
                    nblk = 4 * j + 4  # causal: tk blocks 0 .. 4j+3
                    accA = psum.tile([D + 1, 512], F32, name="acc", bufs=2)
                    accB = psum.tile([D + 1, 512], F32, name="acc", bufs=2)
                    pend = []  # software pipeline: AV for block i-1 after S of i

                    def flush_av():
                        for mm in pend:
                            nc.tensor.matmul(**mm)
                        pend.clear()

                    for i in range(nblk):
                        tk = slice(128 * i, 128 * (i + 1))
                        diag = i - 4 * j
                        lo = 128 * diag if diag >= 0 else 0
                        tqs = slice(tq0 + lo, tq0 + 512)
                        sps = psum.tile([128, 2, 512], F32, name="sps", bufs=2)
                        for h2, lohi in ((0, slice(0, 64)), (1, slice(64, 128))):
                            nc.tensor.matmul(
                                sps[:, h2, lo:],
                                lhsT=kt_t[lohi, hp, tk],
                                rhs=qt_t[lohi, hp, tqs],
                                start=True,
                                stop=True,
                            )
                        pt = pt_pool.tile([128, 2, 512], DT, name="pt")
                        nc.scalar.activation(
                            pt[:, :, lo:],
                            sps[:, :, lo:],
                            mybir.ActivationFunctionType.Exp,
                            scale=0.125,
                        )
                        if diag >= 0:  # zero the above-diagonal triangle
                            dg = slice(lo, lo + 128)
                            nc.vector.tensor_mul(
                                pt[:, :, dg], pt[:, :, dg], tri_t[:, :, :]
                            )
                        if hp < MD - 1:
                            consume(2)
                        elif j >= 1:
                            # only proj tiles for tq chunks < j have their yt
                            # fully emitted (8 steps per tile, 4 tiles/chunk)
                            consume_proj(2, cap=8 * 4 * j)
                        flush_av()
                        for h2, acc in ((0, accA), (1, accB)):
                            pend.append(
                                dict(
                                    out=acc[:, lo:],
                                    lhsT=v_t[:, i, 2 * hp + h2, :],
                                    rhs=pt[:, h2, lo:],
                                    start=(i == 0),
                                    stop=(i == nblk - 1),
                                )
                            )
                    flush_av()

                    # normalization tail. Order matters: the psum-reading
                    # copies (yu, lb) for BOTH heads go first so the acc psum
                    # banks free quickly for the next j / next stage.
                    tq = slice(tq0, tq0 + 512)
                    yus, lbs = [], []
                    for h2, acc in ((0, accA), (1, accB)):
                        yu = yu_pool.tile([64, 512], DT, name="yu")
                        nc.vector.tensor_copy(yu[:, :], acc[0:D, :])
                        lb = rl_pool.tile([1, 512], F32, name="lb")
                        nc.vector.tensor_copy(lb[:, :], acc[D : D + 1, :])
                        yus.append(yu)
                        lbs.append(lb)
                    for h2 in range(2):
                        lbc = rl_pool.tile([64, 512], F32, name="lbc")
                        nc.gpsimd.partition_broadcast(lbc[:, :], lbs[h2][:, :])
                        rli = rl_pool.tile([64, 512], F32, name="rli")
                        nc.vector.reciprocal_approx_fast(rli[:, :], lbc[:, :])
                        nc.vector.tensor_mul(
                            yt_t[64 * h2 : 64 * (h2 + 1), hp, tq],
                            yus[h2][:, :],
                            rli[:, :],
                        )

            # ---- main pipeline ----
            # prologue ordered by DMA arrival: q(j0) [wq,x0], v(0-3) [wv],
            # k(j0) [wk]; the rest of stage-0 q/k flows through the fillers
            warmup(N_WARM)
            for _ in qk_steps(0, [0], parts=("q",)):
                pass
            warmup(N_WARM2)
            for _ in v_steps(0, 4):
                pass
            for _ in qk_steps(0, [0], parts=("k",)):
                pass

            for hp in range(MD):
                if hp >= 1:
                    drain(f"qk{hp}")
                if hp == MD - 1:
                    proj.update(it=proj_steps(), done=False)
                attn_stage(hp)

            consume_proj(10**9)  # finish remaining projection tiles

    nc.compile()
    return nc


def make_host_inputs(x, w_qkv, w_proj, t_len=T):
    """Shard full inputs into the 8 per-core input dicts.

    Everything is pre-arranged into the on-chip SBUF layout
    [partition, chunk, free] (contiguous) so each DMA is one large-run copy.
    """
    NJ = t_len // 512
    tri = np.where(
        np.arange(128)[None, :] >= np.arange(128)[:, None], 1.0, 0.0
    ).astype(BF16)

    def chunked(w, width):  # [C, width] -> [128, C//128, width] contiguous
        return np.ascontiguousarray(
            w.reshape(C // 128, 128, width).transpose(1, 0, 2)
        ).astype(BF16)

    in_maps = []
    for c in range(8):
        b, g = c // 2, c % 2
        xT = np.ascontiguousarray(x[b][:t_len].T)  # [C, T]
        xq = xT.reshape(CK, 128, t_len)
        m = dict(
            wq=chunked(w_qkv[:, 512 * g : 512 * (g + 1)], DL),
            wk=chunked(w_qkv[:, C + 512 * g : C + 512 * (g + 1)], DL),
            wv=chunked(w_qkv[:, 2 * C + 512 * g : 2 * C + 512 * (g + 1)], DL),
            wo=np.ascontiguousarray(
                w_proj[512 * g : 512 * (g + 1), :]
                .reshape(MD := DL // 128, 128, C)
                .transpose(1, 0, 2)
            ).astype(BF16),
            tri=tri,
        )
        for q in range(NJ):
            m[f"x{q}"] = np.ascontiguousarray(
                xq[:, :, 512 * q : 512 * (q + 1)].transpose(1, 0, 2)
            ).astype(BF16)
        in_maps.append(m)
    return in_maps


_CACHE = {}


def _get_program():
    if "nc" not in _CACHE:
        _CACHE["nc"] = build_program()
    return _CACHE["nc"]


def kernel(x, w_qkv, w_proj, _trace=False, _trace_kwargs=None):
    x = np.asarray(x, np.float32)
    w_qkv = np.asarray(w_qkv, np.float32)
    w_proj = np.asarray(w_proj, np.float32)
    nc = _get_program()
    in_maps = make_host_inputs(x, w_qkv, w_proj)
    kw = {}
    if _trace:
        kw = dict(trace=True, **(_trace_kwargs or {}))
    res = run_bass_kernel_spmd(nc, in_maps, core_ids=list(range(8)), **kw)
    out = np.empty((B, T, C), np.float32)
    for b in range(B):
        out[b] = res.results[2 * b]["out"].astype(np.float32) + res.results[
            2 * b + 1
        ]["out"].astype(np.float32)
    if _trace:
        return out, res
    return out
